# revision 15
# baseline (speedup 1.0000x reference)
"""Capsule-routing (ClassCapsLayer) Bass/Tile kernel for 8 trn2 NeuronCores.

Math (reference):
    priors[b,c,r,o] = sum_i x[b,c,r,i] * w[c,r,i,o]
    logits_1 = 0;  logits_{t+1} = logits_t + priors * v_t
    probs_t = softmax_r(logits_t);  s_t = sum_r probs_t * priors
    v_t = squash(s_t)  with GLOBAL Frobenius norm n2 = sum(s_t^2) over (b,c,o)

Key identity: logits_t = priors * W_t with W_t = sum_{u<t} v_u, a per-(b,c,o)
scalar. So each routing iteration needs only one ACT pass
(e = exp(W*priors), fused per-partition scale + fused denominator reduce) and
one DVE pass (tensor_tensor_reduce: numerator = sum_r e*priors), if priors are
laid out with (route-half, o) on partitions and the route index on the free dim.

Matmul: per (class, route-pair) the stationary operand is a 128x128
block-diagonal bf16 weight tile (two 64x64 route weight blocks) -> output
partitions = (half, o), FWL-eligible; moving operand is x [128, B=8].

Sharding: classes split 4-per-core (weights are read exactly once fleet-wide).
The only cross-core quantity is the scalar n2 per iteration -> AllReduce of a
single f32. The final squash is done on the host from per-core partial
numerators/denominators.
"""

import numpy as np
import ml_dtypes

import concourse.bass as bass
import concourse.tile as tile
from concourse import bacc, mybir
from concourse.bass import ts
from concourse.bass_utils import run_bass_kernel_spmd

# Full problem dims (hardcoded; kernel.py must be self-contained)
B, C, R, I, O = 8, 32, 2048, 64, 64
NCORES = 8
CL = C // NCORES      # classes per core
G = 64                # route-pair groups per DMA batch
P = 128

F32 = mybir.dt.float32
BF16 = mybir.dt.bfloat16
AF = mybir.ActivationFunctionType
ALU = mybir.AluOpType

TRACE = False         # set by test.py to collect HW exec time
TMPDIR = None         # set by test.py to keep NTFF/perfetto artifacts
LAST_RESULT = [None]  # BassKernelResults of the most recent run

_cache = {}


def build(iters, cl=CL, rh=R // 2, g_batch=G, b_dim=B, ncores=NCORES):
    """Build the SPMD program. rh = routes/2 (route-pair index range)."""
    nb = rh // g_batch
    nc = bacc.Bacc(
        "TRN2", target_bir_lowering=False, debug=False, num_devices=ncores
    )
    w_in = nc.dram_tensor(
        "w_in", [cl, 2, nb, 64, g_batch, 64], BF16, kind="ExternalInput"
    ).ap()
    x_in = nc.dram_tensor(
        "x_in", [cl, nb, P, g_batch, b_dim], BF16, kind="ExternalInput"
    ).ap()
    f2_in = nc.dram_tensor("f2_in", [P, P], F32, kind="ExternalInput").ap()
    onek_in = nc.dram_tensor("onek_in", [P, 1], F32, kind="ExternalInput").ap()
    onem_in = nc.dram_tensor("onem_in", [1, P], F32, kind="ExternalInput").ap()
    num_o = nc.dram_tensor("num_o", [P, cl, b_dim], F32, kind="ExternalOutput").ap()
    den_o = nc.dram_tensor("den_o", [P, cl, b_dim], F32, kind="ExternalOutput").ap()

    with tile.TileContext(nc) as tc:
        with (
            tc.tile_pool(name="persist", bufs=1) as persist,
            tc.tile_pool(name="wpool", bufs=2) as wpool,
            tc.tile_pool(name="xpool", bufs=3) as xpool,
            tc.tile_pool(name="ppool", bufs=2, space="PSUM") as ppool,
            tc.tile_pool(name="psmall", bufs=1, space="PSUM") as psmall,
            tc.tile_pool(name="scratch", bufs=2) as scratch,
            tc.tile_pool(name="dram", bufs=2, space="DRAM") as dram,
        ):
            # ---- persistent state ----
            # b-major so each (c,b) routing tile is a contiguous [P, rh] slice
            priors = persist.tile([P, cl, b_dim, rh], F32)
            f2_sb = persist.tile([P, P], F32)
            nc.sync.dma_start(f2_sb[:], f2_in[:])
            onek_sb = persist.tile([P, 1], F32)
            nc.sync.dma_start(onek_sb[:], onek_in[:])
            onem_sb = persist.tile([1, P], F32)
            nc.sync.dma_start(onem_sb[:], onem_in[:])
            w_t = persist.tile([P, cl, b_dim], F32)
            nc.vector.memset(w_t[:], 0.0)

            # ---- priors matmul ----
            # Dense stationary: partitions 0-63 hold route-half-0 weights,
            # 64-127 hold half-1. Each group issues two 64x64-stationary
            # matmuls on disjoint row/col groups of the PE array
            # (tile_position), so their LDWEIGHTS/MATMULs overlap and the
            # outputs land at partitions 0-63 / 64-127 of the same bank.
            for c in range(cl):
                for n in range(nb):
                    wb = wpool.tile([P, g_batch, 64], BF16, tag="wb")
                    dma_eng = nc.sync if (c * nb + n) % 2 == 0 else nc.gpsimd
                    dma_eng.dma_start(wb[:], w_in[c, :, n])
                    xs = xpool.tile([P, g_batch, b_dim], BF16, tag="xs")
                    nc.scalar.dma_start(xs[:], x_in[c, n])
                    pt = ppool.tile([P, g_batch, b_dim], F32, tag="pt")
                    for gi in range(g_batch):
                        nc.tensor.matmul(
                            pt[0:64, gi],
                            wb[0:64, gi, :],
                            xs[0:64, gi],
                            start=True,
                            stop=True,
                        )
                        nc.tensor.matmul(
                            pt[64:128, gi],
                            wb[64:128, gi, :],
                            xs[64:128, gi],
                            start=True,
                            stop=True,
                            tile_position=(64, 64),
                        )
                    nc.vector.tensor_copy(
                        priors[:, c, :, ts(n, g_batch)].rearrange(
                            "p b g -> p g b"
                        ),
                        pt[:],
                    )

            # ---- routing iterations ----
            for it in range(iters):
                num_t = scratch.tile([P, cl, b_dim], F32, tag="num")
                den_t = scratch.tile([P, cl, b_dim], F32, tag="den")
                k = 0
                for c in range(cl):
                    for b in range(b_dim):
                        pr = priors[:, c, b, :]  # [P, rh] contiguous
                        if it == 0:
                            # W == 0 -> e == 1: den is a constant, num is a
                            # plain reduction of priors (split ACT/DVE).
                            if k == 0:
                                nc.vector.memset(den_t[:], float(rh))
                            if k % 2 == 0:
                                nc.vector.tensor_reduce(
                                    num_t[:, c, b : b + 1],
                                    pr,
                                    mybir.AxisListType.X,
                                    ALU.add,
                                )
                            else:
                                sc_t = scratch.tile([P, rh], F32, tag="sc")
                                nc.scalar.activation(
                                    sc_t[:],
                                    pr,
                                    AF.Copy,
                                    accum_out=num_t[:, c, b : b + 1],
                                )
                        else:
                            # e = exp(W * priors); den += sum_r e
                            e_t = scratch.tile([P, rh], F32, tag="e")
                            nc.scalar.activation(
                                e_t[:],
                                pr,
                                AF.Exp,
                                scale=w_t[:, c, b : b + 1],
                                accum_out=den_t[:, c, b : b + 1],
                            )
                            # num = sum_r e * priors (mul on DVE; the
                            # reduction is load-balanced ACT/DVE ~5:3)
                            t_t = scratch.tile([P, rh], F32, tag="tt")
                            nc.vector.tensor_mul(t_t[:], e_t[:], pr)
                            if k % 2 == 0:
                                nc.vector.tensor_reduce(
                                    num_t[:, c, b : b + 1],
                                    t_t[:],
                                    mybir.AxisListType.X,
                                    ALU.add,
                                )
                            else:
                                sc_t = scratch.tile([P, rh], F32, tag="sc")
                                nc.scalar.activation(
                                    sc_t[:],
                                    t_t[:],
                                    AF.Copy,
                                    accum_out=num_t[:, c, b : b + 1],
                                )
                        k += 1
                if it == iters - 1:
                    nc.sync.dma_start(num_o[:], num_t[:])
                    nc.sync.dma_start(den_o[:], den_t[:])
                else:
                    # fold the two route-halves (and duplicate into both
                    # halves) with F2[k,m] = (k%64 == m%64): PE matmul
                    nf = psmall.tile([P, cl, b_dim], F32, tag="nf")
                    df = psmall.tile([P, cl, b_dim], F32, tag="df")
                    nc.tensor.matmul(nf[:], f2_sb[:], num_t[:], start=True, stop=True)
                    nc.tensor.matmul(df[:], f2_sb[:], den_t[:], start=True, stop=True)
                    # 1/den via exp(-ln(den)) (ACT-native; den > 0)
                    ld_t = scratch.tile([P, cl, b_dim], F32, tag="ld")
                    nc.scalar.activation(ld_t[:], df[:], AF.Ln)
                    rd_t = scratch.tile([P, cl, b_dim], F32, tag="rd")
                    nc.scalar.activation(rd_t[:], ld_t[:], AF.Exp, scale=-1.0)
                    s_t = scratch.tile([P, cl, b_dim], F32, tag="s")
                    nc.vector.tensor_mul(s_t[:], nf[:], rd_t[:])
                    # n2_partial = sum(s^2)/2 (each value appears in both halves)
                    sq_t = scratch.tile([P, cl, b_dim], F32, tag="sq")
                    sacc = scratch.tile([P, 1], F32, tag="sacc")
                    nc.scalar.activation(
                        sq_t[:], s_t[:], AF.Square, accum_out=sacc[:]
                    )
                    n2p = psmall.tile([1, 1], F32, tag="n2p")
                    nc.tensor.matmul(n2p[:], onek_sb[:], sacc[:], start=True, stop=True)
                    n2sb = scratch.tile([1, 1], F32, tag="n2sb")
                    nc.any.tensor_copy(n2sb[:], n2p[:])
                    cc_in = dram.tile([1, 1], F32, tag="ccin")
                    cc_out = dram.tile([1, 1], F32, tag="ccout")
                    nc.gpsimd.dma_start(cc_in[:], n2sb[:])
                    nc.gpsimd.collective_compute(
                        "AllReduce",
                        ALU.add,
                        replica_groups=[list(range(ncores))],
                        ins=[cc_in.opt()],
                        outs=[cc_out.opt()],
                    )
                    n2g = scratch.tile([1, 1], F32, tag="n2g")
                    nc.gpsimd.dma_start(n2g[:], cc_out[:])
                    # squash scale g = sqrt(n2)/(1+n2), n2 = 0.5*allreduced
                    r_t = scratch.tile([1, 1], F32, tag="rt")
                    nc.scalar.activation(r_t[:], n2g[:], AF.Sqrt, scale=0.5)
                    t1_t = scratch.tile([1, 1], F32, tag="t1")
                    nc.vector.tensor_scalar(
                        t1_t[:], n2g[:], 0.5, 1.0, ALU.mult, ALU.add
                    )
                    lt1 = scratch.tile([1, 1], F32, tag="lt1")
                    nc.scalar.activation(lt1[:], t1_t[:], AF.Ln)
                    rt2 = scratch.tile([1, 1], F32, tag="rt2")
                    nc.scalar.activation(rt2[:], lt1[:], AF.Exp, scale=-1.0)
                    g_t = scratch.tile([1, 1], F32, tag="g")
                    nc.vector.tensor_mul(g_t[:], r_t[:], rt2[:])
                    # broadcast g to all partitions via K=1 matmul with ones
                    gb_ps = psmall.tile([P, 1], F32, tag="gb")
                    nc.tensor.matmul(gb_ps[:], onem_sb[:], g_t[:], start=True, stop=True)
                    gb_sb = scratch.tile([P, 1], F32, tag="gbs")
                    nc.any.tensor_copy(gb_sb[:], gb_ps[:])
                    # v = g*s ; W += v
                    v_t = scratch.tile([P, cl, b_dim], F32, tag="v")
                    nc.vector.tensor_scalar_mul(v_t[:], s_t[:], gb_sb[:])
                    nc.vector.tensor_add(w_t[:], w_t[:], v_t[:])

    nc.compile()
    return nc


def prep_inputs(x, w, cl=CL, rh=R // 2, g_batch=G, b_dim=B, ncores=NCORES):
    """Host-side relayout (f32 -> bf16, DMA-friendly order). Returns in_maps."""
    nb = rh // g_batch
    ctot = cl * ncores
    # w: [C, R, I, O] -> [C, 2, NB, I, G, O] bf16
    wb = (
        w.reshape(ctot, 2, nb, g_batch, 64, 64)
        .transpose(0, 1, 2, 4, 3, 5)
        .astype(ml_dtypes.bfloat16)
    )
    # x: [B, C, R, 1, I] -> [C, NB, (2,I)=128, G, B] bf16
    xb = (
        x.reshape(b_dim, ctot, 2, nb, g_batch, 64)
        .transpose(1, 3, 2, 5, 4, 0)
        .reshape(ctot, nb, P, g_batch, b_dim)
        .astype(ml_dtypes.bfloat16)
    )
    f2 = np.equal.outer(np.arange(P) % 64, np.arange(P) % 64).astype(np.float32)
    onek = np.ones((P, 1), np.float32)
    onem = np.ones((1, P), np.float32)
    in_maps = []
    for k in range(ncores):
        in_maps.append(
            {
                "w_in": np.ascontiguousarray(wb[k * cl : (k + 1) * cl]),
                "x_in": np.ascontiguousarray(xb[k * cl : (k + 1) * cl]),
                "f2_in": f2,
                "onek_in": onek,
                "onem_in": onem,
            }
        )
    return in_maps


def postprocess(results, cl=CL, b_dim=B, ncores=NCORES):
    """Fold halves, divide, global squash -> v [B, C, 1, 1, O] f32."""
    ctot = cl * ncores
    s = np.empty((b_dim, ctot, 64), np.float32)
    for k in range(ncores):
        num = np.asarray(results[k]["num_o"], np.float32)  # [P, cl, B]
        den = np.asarray(results[k]["den_o"], np.float32)
        sk = (num[:64] + num[64:]) / (den[:64] + den[64:])  # [64(o), cl, B]
        s[:, k * cl : (k + 1) * cl, :] = sk.transpose(2, 1, 0)
    n2 = np.sum(s.astype(np.float32) ** 2, dtype=np.float32)
    g = np.float32(np.sqrt(n2) / (1.0 + n2))
    v = (g * s).astype(np.float32)
    return v[:, :, None, None, :]


def kernel(x, route_weights, iterations):
    iters = int(iterations)
    assert iters >= 1
    x = np.asarray(x, dtype=np.float32)
    w = np.asarray(route_weights, dtype=np.float32)
    if iters not in _cache:
        _cache[iters] = build(iters)
    nc = _cache[iters]
    in_maps = prep_inputs(x, w)
    res = run_bass_kernel_spmd(
        nc, in_maps, list(range(NCORES)), trace=TRACE, tmpdir=TMPDIR
    )
    LAST_RESULT[0] = res
    return postprocess(res.results)


# revision 21
# speedup vs baseline: 1.5648x; 1.5648x over previous
"""Capsule-routing (ClassCapsLayer) Bass/Tile kernel for 8 trn2 NeuronCores.

Math (reference):
    priors[b,c,r,o] = sum_i x[b,c,r,i] * w[c,r,i,o]
    logits_1 = 0;  logits_{t+1} = logits_t + priors * v_t
    probs_t = softmax_r(logits_t);  s_t = sum_r probs_t * priors
    v_t = squash(s_t)  with GLOBAL Frobenius norm n2 = sum(s_t^2) over (b,c,o)

Key identity: logits_t = priors * W_t with W_t = sum_{u<t} v_u, a per-(b,c,o)
scalar. So each routing iteration needs only one ACT pass
(e = exp(W*priors), fused per-partition scale + fused denominator reduce) and
one DVE pass (tensor_tensor_reduce: numerator = sum_r e*priors), if priors are
laid out with (route-half, o) on partitions and the route index on the free dim.

Matmul: per (class, route-pair) the stationary operand is a 128x128
block-diagonal bf16 weight tile (two 64x64 route weight blocks) -> output
partitions = (half, o), FWL-eligible; moving operand is x [128, B=8].

Sharding: classes split 4-per-core (weights are read exactly once fleet-wide).
The only cross-core quantity is the scalar n2 per iteration -> AllReduce of a
single f32. The final squash is done on the host from per-core partial
numerators/denominators.
"""

import numpy as np
import ml_dtypes

import concourse.bass as bass
import concourse.tile as tile
from concourse import bacc, mybir
from concourse.bass import ts
from concourse.bass_utils import run_bass_kernel_spmd

# Full problem dims (hardcoded; kernel.py must be self-contained)
B, C, R, I, O = 8, 32, 2048, 64, 64
NCORES = 8
CL = C // NCORES      # classes per core
G = 64                # route-pair groups per DMA batch
P = 128

F32 = mybir.dt.float32
BF16 = mybir.dt.bfloat16
AF = mybir.ActivationFunctionType
ALU = mybir.AluOpType

TRACE = False         # set by test.py to collect HW exec time
TMPDIR = None         # set by test.py to keep NTFF/perfetto artifacts
LAST_RESULT = [None]  # BassKernelResults of the most recent run

_cache = {}


def build(iters, cl=CL, rh=R // 2, g_batch=G, b_dim=B, ncores=NCORES):
    """Build the SPMD program. rh = routes/2 (route-pair index range)."""
    nb = rh // g_batch
    nc = bacc.Bacc(
        "TRN2", target_bir_lowering=False, debug=False, num_devices=ncores
    )
    w_in = nc.dram_tensor(
        "w_in", [cl, 2, nb, 64, g_batch, 64], BF16, kind="ExternalInput"
    ).ap()
    x_in = nc.dram_tensor(
        "x_in", [cl, nb, P, g_batch, b_dim], BF16, kind="ExternalInput"
    ).ap()
    f2_in = nc.dram_tensor("f2_in", [P, P], F32, kind="ExternalInput").ap()
    onek_in = nc.dram_tensor("onek_in", [P, 1], F32, kind="ExternalInput").ap()
    onem_in = nc.dram_tensor("onem_in", [1, P], F32, kind="ExternalInput").ap()
    num_o = nc.dram_tensor("num_o", [P, cl, b_dim], F32, kind="ExternalOutput").ap()
    den_o = nc.dram_tensor("den_o", [P, cl, b_dim], F32, kind="ExternalOutput").ap()

    with tile.TileContext(nc) as tc:
        with (
            tc.tile_pool(name="persist", bufs=1) as persist,
            tc.tile_pool(name="wpool", bufs=2) as wpool,
            tc.tile_pool(name="xpool", bufs=3) as xpool,
            tc.tile_pool(name="ppool", bufs=2, space="PSUM") as ppool,
            tc.tile_pool(name="psmall", bufs=1, space="PSUM") as psmall,
            tc.tile_pool(name="scratch", bufs=2) as scratch,
            tc.tile_pool(name="dram", bufs=2, space="DRAM") as dram,
        ):
            # ---- persistent state ----
            # b-major so each (c,b) routing tile is a contiguous [P, rh] slice
            priors = persist.tile([P, cl, b_dim, rh], F32)
            f2_sb = persist.tile([P, P], F32)
            nc.sync.dma_start(f2_sb[:], f2_in[:])
            onek_sb = persist.tile([P, 1], F32)
            nc.sync.dma_start(onek_sb[:], onek_in[:])
            onem_sb = persist.tile([1, P], F32)
            nc.sync.dma_start(onem_sb[:], onem_in[:])
            w_t = persist.tile([P, cl, b_dim], F32)
            nc.vector.memset(w_t[:], 0.0)

            # Two persistent block-diagonal stationary buffers, zeroed once;
            # per-batch DMAs only write the diagonal quadrants, so the
            # off-diagonal zeros persist. Alternating gives double-buffering.
            wb_slots = []
            for si in range(2):
                wbs = persist.tile([P, g_batch, P], BF16, tag=f"wb{si}")
                nc.vector.memset(wbs[:], 0.0)
                wb_slots.append(wbs)

            # ---- priors matmul ----
            # Quadrant DMAs are 128B-line strided; spread them over four
            # HWDGE queues (two per quadrant stream) to parallelize.
            top_eng = [nc.sync, nc.scalar]
            bot_eng = [nc.gpsimd, nc.gpsimd]
            for c in range(cl):
                for n in range(nb):
                    bi = c * nb + n
                    wb = wb_slots[bi % 2]
                    top_eng[bi % 2].dma_start(wb[0:64, :, 0:64], w_in[c, 0, n])
                    bot_eng[bi % 2].dma_start(wb[64:128, :, 64:128], w_in[c, 1, n])
                    xs = xpool.tile([P, g_batch, b_dim], BF16, tag="xs")
                    (top_eng + bot_eng)[bi % 4].dma_start(xs[:], x_in[c, n])
                    pt = ppool.tile([P, g_batch, b_dim], F32, tag="pt")
                    for gi in range(g_batch):
                        # out[(h,o), b] = blockdiag_w[(h,i),(h,o)] @ x[(h,i), b]
                        nc.tensor.matmul(
                            pt[:, gi],
                            wb[:, gi, :],
                            xs[:, gi],
                            start=True,
                            stop=True,
                        )
                    nc.vector.tensor_copy(
                        priors[:, c, :, ts(n, g_batch)].rearrange(
                            "p b g -> p g b"
                        ),
                        pt[:],
                    )

            # ---- routing iterations ----
            for it in range(iters):
                num_t = scratch.tile([P, cl, b_dim], F32, tag="num")
                den_t = scratch.tile([P, cl, b_dim], F32, tag="den")
                k = 0
                for c in range(cl):
                    for b in range(b_dim):
                        pr = priors[:, c, b, :]  # [P, rh] contiguous
                        if it == 0:
                            # W == 0 -> e == 1: den is a constant, num is a
                            # plain reduction of priors (split ACT/DVE).
                            if k == 0:
                                nc.vector.memset(den_t[:], float(rh))
                            if k % 2 == 0:
                                nc.vector.tensor_reduce(
                                    num_t[:, c, b : b + 1],
                                    pr,
                                    mybir.AxisListType.X,
                                    ALU.add,
                                )
                            else:
                                sc_t = scratch.tile([P, rh], F32, tag="sc")
                                nc.scalar.activation(
                                    sc_t[:],
                                    pr,
                                    AF.Copy,
                                    accum_out=num_t[:, c, b : b + 1],
                                )
                        else:
                            # e = exp(W * priors); den += sum_r e
                            e_t = scratch.tile([P, rh], F32, tag="e")
                            nc.scalar.activation(
                                e_t[:],
                                pr,
                                AF.Exp,
                                scale=w_t[:, c, b : b + 1],
                                accum_out=den_t[:, c, b : b + 1],
                            )
                            # num = sum_r e * priors (mul on DVE; the
                            # reduction is load-balanced ACT/DVE ~5:3)
                            t_t = scratch.tile([P, rh], F32, tag="tt")
                            nc.vector.tensor_mul(t_t[:], e_t[:], pr)
                            if k % 2 == 0:
                                nc.vector.tensor_reduce(
                                    num_t[:, c, b : b + 1],
                                    t_t[:],
                                    mybir.AxisListType.X,
                                    ALU.add,
                                )
                            else:
                                sc_t = scratch.tile([P, rh], F32, tag="sc")
                                nc.scalar.activation(
                                    sc_t[:],
                                    t_t[:],
                                    AF.Copy,
                                    accum_out=num_t[:, c, b : b + 1],
                                )
                        k += 1
                if it == iters - 1:
                    nc.sync.dma_start(num_o[:], num_t[:])
                    nc.sync.dma_start(den_o[:], den_t[:])
                else:
                    # fold the two route-halves (and duplicate into both
                    # halves) with F2[k,m] = (k%64 == m%64): PE matmul
                    nf = psmall.tile([P, cl, b_dim], F32, tag="nf")
                    df = psmall.tile([P, cl, b_dim], F32, tag="df")
                    nc.tensor.matmul(nf[:], f2_sb[:], num_t[:], start=True, stop=True)
                    nc.tensor.matmul(df[:], f2_sb[:], den_t[:], start=True, stop=True)
                    # 1/den via exp(-ln(den)) (ACT-native; den > 0)
                    ld_t = scratch.tile([P, cl, b_dim], F32, tag="ld")
                    nc.scalar.activation(ld_t[:], df[:], AF.Ln)
                    rd_t = scratch.tile([P, cl, b_dim], F32, tag="rd")
                    nc.scalar.activation(rd_t[:], ld_t[:], AF.Exp, scale=-1.0)
                    s_t = scratch.tile([P, cl, b_dim], F32, tag="s")
                    nc.vector.tensor_mul(s_t[:], nf[:], rd_t[:])
                    # n2_partial = sum(s^2)/2 (each value appears in both halves)
                    sq_t = scratch.tile([P, cl, b_dim], F32, tag="sq")
                    sacc = scratch.tile([P, 1], F32, tag="sacc")
                    nc.scalar.activation(
                        sq_t[:], s_t[:], AF.Square, accum_out=sacc[:]
                    )
                    n2p = psmall.tile([1, 1], F32, tag="n2p")
                    nc.tensor.matmul(n2p[:], onek_sb[:], sacc[:], start=True, stop=True)
                    n2sb = scratch.tile([1, 1], F32, tag="n2sb")
                    nc.any.tensor_copy(n2sb[:], n2p[:])
                    cc_in = dram.tile([1, 1], F32, tag="ccin")
                    cc_out = dram.tile([1, 1], F32, tag="ccout")
                    nc.gpsimd.dma_start(cc_in[:], n2sb[:])
                    nc.gpsimd.collective_compute(
                        "AllReduce",
                        ALU.add,
                        replica_groups=[list(range(ncores))],
                        ins=[cc_in.opt()],
                        outs=[cc_out.opt()],
                    )
                    n2g = scratch.tile([1, 1], F32, tag="n2g")
                    nc.gpsimd.dma_start(n2g[:], cc_out[:])
                    # squash scale g = sqrt(n2)/(1+n2), n2 = 0.5*allreduced
                    r_t = scratch.tile([1, 1], F32, tag="rt")
                    nc.scalar.activation(r_t[:], n2g[:], AF.Sqrt, scale=0.5)
                    t1_t = scratch.tile([1, 1], F32, tag="t1")
                    nc.vector.tensor_scalar(
                        t1_t[:], n2g[:], 0.5, 1.0, ALU.mult, ALU.add
                    )
                    lt1 = scratch.tile([1, 1], F32, tag="lt1")
                    nc.scalar.activation(lt1[:], t1_t[:], AF.Ln)
                    rt2 = scratch.tile([1, 1], F32, tag="rt2")
                    nc.scalar.activation(rt2[:], lt1[:], AF.Exp, scale=-1.0)
                    g_t = scratch.tile([1, 1], F32, tag="g")
                    nc.vector.tensor_mul(g_t[:], r_t[:], rt2[:])
                    # broadcast g to all partitions via K=1 matmul with ones
                    gb_ps = psmall.tile([P, 1], F32, tag="gb")
                    nc.tensor.matmul(gb_ps[:], onem_sb[:], g_t[:], start=True, stop=True)
                    gb_sb = scratch.tile([P, 1], F32, tag="gbs")
                    nc.any.tensor_copy(gb_sb[:], gb_ps[:])
                    # v = g*s ; W += v
                    v_t = scratch.tile([P, cl, b_dim], F32, tag="v")
                    nc.vector.tensor_scalar_mul(v_t[:], s_t[:], gb_sb[:])
                    nc.vector.tensor_add(w_t[:], w_t[:], v_t[:])

    nc.compile()
    return nc


def prep_inputs(x, w, cl=CL, rh=R // 2, g_batch=G, b_dim=B, ncores=NCORES):
    """Host-side relayout (f32 -> bf16, DMA-friendly order). Returns in_maps."""
    nb = rh // g_batch
    ctot = cl * ncores
    # w: [C, R, I, O] -> [C, 2, NB, I, G, O] bf16
    wb = (
        w.reshape(ctot, 2, nb, g_batch, 64, 64)
        .transpose(0, 1, 2, 4, 3, 5)
        .astype(ml_dtypes.bfloat16)
    )
    # x: [B, C, R, 1, I] -> [C, NB, (2,I)=128, G, B] bf16
    xb = (
        x.reshape(b_dim, ctot, 2, nb, g_batch, 64)
        .transpose(1, 3, 2, 5, 4, 0)
        .reshape(ctot, nb, P, g_batch, b_dim)
        .astype(ml_dtypes.bfloat16)
    )
    f2 = np.equal.outer(np.arange(P) % 64, np.arange(P) % 64).astype(np.float32)
    onek = np.ones((P, 1), np.float32)
    onem = np.ones((1, P), np.float32)
    in_maps = []
    for k in range(ncores):
        in_maps.append(
            {
                "w_in": np.ascontiguousarray(wb[k * cl : (k + 1) * cl]),
                "x_in": np.ascontiguousarray(xb[k * cl : (k + 1) * cl]),
                "f2_in": f2,
                "onek_in": onek,
                "onem_in": onem,
            }
        )
    return in_maps


def postprocess(results, cl=CL, b_dim=B, ncores=NCORES):
    """Fold halves, divide, global squash -> v [B, C, 1, 1, O] f32."""
    ctot = cl * ncores
    s = np.empty((b_dim, ctot, 64), np.float32)
    for k in range(ncores):
        num = np.asarray(results[k]["num_o"], np.float32)  # [P, cl, B]
        den = np.asarray(results[k]["den_o"], np.float32)
        sk = (num[:64] + num[64:]) / (den[:64] + den[64:])  # [64(o), cl, B]
        s[:, k * cl : (k + 1) * cl, :] = sk.transpose(2, 1, 0)
    n2 = np.sum(s.astype(np.float32) ** 2, dtype=np.float32)
    g = np.float32(np.sqrt(n2) / (1.0 + n2))
    v = (g * s).astype(np.float32)
    return v[:, :, None, None, :]


def kernel(x, route_weights, iterations):
    iters = int(iterations)
    assert iters >= 1
    x = np.asarray(x, dtype=np.float32)
    w = np.asarray(route_weights, dtype=np.float32)
    if iters not in _cache:
        _cache[iters] = build(iters)
    nc = _cache[iters]
    in_maps = prep_inputs(x, w)
    res = run_bass_kernel_spmd(
        nc, in_maps, list(range(NCORES)), trace=TRACE, tmpdir=TMPDIR
    )
    LAST_RESULT[0] = res
    return postprocess(res.results)


# revision 23
# speedup vs baseline: 1.5825x; 1.0113x over previous
"""Capsule-routing (ClassCapsLayer) Bass/Tile kernel for 8 trn2 NeuronCores.

Math (reference):
    priors[b,c,r,o] = sum_i x[b,c,r,i] * w[c,r,i,o]
    logits_1 = 0;  logits_{t+1} = logits_t + priors * v_t
    probs_t = softmax_r(logits_t);  s_t = sum_r probs_t * priors
    v_t = squash(s_t)  with GLOBAL Frobenius norm n2 = sum(s_t^2) over (b,c,o)

Key identity: logits_t = priors * W_t with W_t = sum_{u<t} v_u, a per-(b,c,o)
scalar. So each routing iteration needs only one ACT pass
(e = exp(W*priors), fused per-partition scale + fused denominator reduce) and
one DVE pass (tensor_tensor_reduce: numerator = sum_r e*priors), if priors are
laid out with (route-half, o) on partitions and the route index on the free dim.

Matmul: per (class, route-pair) the stationary operand is a 128x128
block-diagonal bf16 weight tile (two 64x64 route weight blocks) -> output
partitions = (half, o), FWL-eligible; moving operand is x [128, B=8].

Sharding: classes split 4-per-core (weights are read exactly once fleet-wide).
The only cross-core quantity is the scalar n2 per iteration -> AllReduce of a
single f32. The final squash is done on the host from per-core partial
numerators/denominators.
"""

import numpy as np
import ml_dtypes

import concourse.bass as bass
import concourse.tile as tile
from concourse import bacc, mybir
from concourse.bass import ts
from concourse.bass_utils import run_bass_kernel_spmd

# Full problem dims (hardcoded; kernel.py must be self-contained)
B, C, R, I, O = 8, 32, 2048, 64, 64
NCORES = 8
CL = C // NCORES      # classes per core
G = 64                # route-pair groups per DMA batch
P = 128

F32 = mybir.dt.float32
BF16 = mybir.dt.bfloat16
AF = mybir.ActivationFunctionType
ALU = mybir.AluOpType

TRACE = False         # set by test.py to collect HW exec time
TMPDIR = None         # set by test.py to keep NTFF/perfetto artifacts
LAST_RESULT = [None]  # BassKernelResults of the most recent run

_cache = {}


def build(iters, cl=CL, rh=R // 2, g_batch=G, b_dim=B, ncores=NCORES):
    """Build the SPMD program. rh = routes/2 (route-pair index range)."""
    nb = rh // g_batch
    nc = bacc.Bacc(
        "TRN2", target_bir_lowering=False, debug=False, num_devices=ncores
    )
    w_in = nc.dram_tensor(
        "w_in", [cl, 2, nb, 64, g_batch, 64], BF16, kind="ExternalInput"
    ).ap()
    x_in = nc.dram_tensor(
        "x_in", [cl, nb, P, g_batch, b_dim], BF16, kind="ExternalInput"
    ).ap()
    f2_in = nc.dram_tensor("f2_in", [P, P], F32, kind="ExternalInput").ap()
    onek_in = nc.dram_tensor("onek_in", [P, 1], F32, kind="ExternalInput").ap()
    onem_in = nc.dram_tensor("onem_in", [1, P], F32, kind="ExternalInput").ap()
    num_o = nc.dram_tensor("num_o", [P, cl, b_dim], F32, kind="ExternalOutput").ap()
    den_o = nc.dram_tensor("den_o", [P, cl, b_dim], F32, kind="ExternalOutput").ap()

    with tile.TileContext(nc) as tc:
        with (
            tc.tile_pool(name="persist", bufs=1) as persist,
            tc.tile_pool(name="wpool", bufs=2) as wpool,
            tc.tile_pool(name="xpool", bufs=3) as xpool,
            tc.tile_pool(name="ppool", bufs=2, space="PSUM") as ppool,
            tc.tile_pool(name="psmall", bufs=1, space="PSUM") as psmall,
            tc.tile_pool(name="scratch", bufs=2) as scratch,
            tc.tile_pool(name="dram", bufs=2, space="DRAM") as dram,
        ):
            # ---- persistent state ----
            # b-major so each (c,b) routing tile is a contiguous [P, rh] slice
            priors = persist.tile([P, cl, b_dim, rh], F32)
            f2_sb = persist.tile([P, P], F32)
            nc.sync.dma_start(f2_sb[:], f2_in[:])
            onek_sb = persist.tile([P, 1], F32)
            nc.sync.dma_start(onek_sb[:], onek_in[:])
            onem_sb = persist.tile([1, P], F32)
            nc.sync.dma_start(onem_sb[:], onem_in[:])
            w_t = persist.tile([P, cl, b_dim], F32)
            nc.vector.memset(w_t[:], 0.0)

            # Two persistent block-diagonal stationary buffers, zeroed once;
            # per-batch DMAs only write the diagonal quadrants, so the
            # off-diagonal zeros persist. Alternating gives double-buffering.
            wb_slots = []
            for si in range(2):
                wbs = persist.tile([P, g_batch, P], BF16, tag=f"wb{si}")
                nc.vector.memset(wbs[:], 0.0)
                wb_slots.append(wbs)

            # ---- priors matmul ----
            # Quadrant DMAs are 128B-line strided; spread them over four
            # HWDGE queues (two per quadrant stream) to parallelize.
            top_eng = [nc.sync, nc.sync]
            bot_eng = [nc.gpsimd, nc.gpsimd]
            for c in range(cl):
                for n in range(nb):
                    bi = c * nb + n
                    wb = wb_slots[bi % 2]
                    top_eng[bi % 2].dma_start(wb[0:64, :, 0:64], w_in[c, 0, n])
                    bot_eng[bi % 2].dma_start(wb[64:128, :, 64:128], w_in[c, 1, n])
                    xs = xpool.tile([P, g_batch, b_dim], BF16, tag="xs")
                    nc.scalar.dma_start(xs[:], x_in[c, n])
                    pt = ppool.tile([P, g_batch, b_dim], F32, tag="pt")
                    for gi in range(g_batch):
                        # out[(h,o), b] = blockdiag_w[(h,i),(h,o)] @ x[(h,i), b]
                        nc.tensor.matmul(
                            pt[:, gi],
                            wb[:, gi, :],
                            xs[:, gi],
                            start=True,
                            stop=True,
                        )
                    nc.vector.tensor_copy(
                        priors[:, c, :, ts(n, g_batch)].rearrange(
                            "p b g -> p g b"
                        ),
                        pt[:],
                    )

            # ---- routing iterations ----
            for it in range(iters):
                num_t = scratch.tile([P, cl, b_dim], F32, tag="num")
                den_t = scratch.tile([P, cl, b_dim], F32, tag="den")
                k = 0
                for c in range(cl):
                    for b in range(b_dim):
                        pr = priors[:, c, b, :]  # [P, rh] contiguous
                        if it == 0:
                            # W == 0 -> e == 1: den is a constant, num is a
                            # plain reduction of priors (split ACT/DVE).
                            if k == 0:
                                nc.vector.memset(den_t[:], float(rh))
                            if k % 2 == 0:
                                nc.vector.tensor_reduce(
                                    num_t[:, c, b : b + 1],
                                    pr,
                                    mybir.AxisListType.X,
                                    ALU.add,
                                )
                            else:
                                sc_t = scratch.tile([P, rh], F32, tag="sc")
                                nc.scalar.activation(
                                    sc_t[:],
                                    pr,
                                    AF.Copy,
                                    accum_out=num_t[:, c, b : b + 1],
                                )
                        else:
                            # e = exp(W * priors); den += sum_r e
                            e_t = scratch.tile([P, rh], F32, tag="e")
                            nc.scalar.activation(
                                e_t[:],
                                pr,
                                AF.Exp,
                                scale=w_t[:, c, b : b + 1],
                                accum_out=den_t[:, c, b : b + 1],
                            )
                            # num = sum_r e * priors (mul on DVE; the
                            # reduction is load-balanced ACT/DVE ~5:3)
                            t_t = scratch.tile([P, rh], F32, tag="tt")
                            nc.vector.tensor_mul(t_t[:], e_t[:], pr)
                            if k % 2 == 0:
                                nc.vector.tensor_reduce(
                                    num_t[:, c, b : b + 1],
                                    t_t[:],
                                    mybir.AxisListType.X,
                                    ALU.add,
                                )
                            else:
                                sc_t = scratch.tile([P, rh], F32, tag="sc")
                                nc.scalar.activation(
                                    sc_t[:],
                                    t_t[:],
                                    AF.Copy,
                                    accum_out=num_t[:, c, b : b + 1],
                                )
                        k += 1
                if it == iters - 1:
                    nc.sync.dma_start(num_o[:], num_t[:])
                    nc.sync.dma_start(den_o[:], den_t[:])
                else:
                    # fold the two route-halves (and duplicate into both
                    # halves) with F2[k,m] = (k%64 == m%64): PE matmul
                    nf = psmall.tile([P, cl, b_dim], F32, tag="nf")
                    df = psmall.tile([P, cl, b_dim], F32, tag="df")
                    nc.tensor.matmul(nf[:], f2_sb[:], num_t[:], start=True, stop=True)
                    nc.tensor.matmul(df[:], f2_sb[:], den_t[:], start=True, stop=True)
                    # 1/den via exp(-ln(den)) (ACT-native; den > 0)
                    ld_t = scratch.tile([P, cl, b_dim], F32, tag="ld")
                    nc.scalar.activation(ld_t[:], df[:], AF.Ln)
                    rd_t = scratch.tile([P, cl, b_dim], F32, tag="rd")
                    nc.scalar.activation(rd_t[:], ld_t[:], AF.Exp, scale=-1.0)
                    s_t = scratch.tile([P, cl, b_dim], F32, tag="s")
                    nc.vector.tensor_mul(s_t[:], nf[:], rd_t[:])
                    # n2_partial = sum(s^2)/2 (each value appears in both halves)
                    sq_t = scratch.tile([P, cl, b_dim], F32, tag="sq")
                    sacc = scratch.tile([P, 1], F32, tag="sacc")
                    nc.scalar.activation(
                        sq_t[:], s_t[:], AF.Square, accum_out=sacc[:]
                    )
                    n2p = psmall.tile([1, 1], F32, tag="n2p")
                    nc.tensor.matmul(n2p[:], onek_sb[:], sacc[:], start=True, stop=True)
                    n2sb = scratch.tile([1, 1], F32, tag="n2sb")
                    nc.any.tensor_copy(n2sb[:], n2p[:])
                    cc_in = dram.tile([1, 1], F32, tag="ccin")
                    cc_out = dram.tile([1, 1], F32, tag="ccout")
                    nc.gpsimd.dma_start(cc_in[:], n2sb[:])
                    nc.gpsimd.collective_compute(
                        "AllReduce",
                        ALU.add,
                        replica_groups=[list(range(ncores))],
                        ins=[cc_in.opt()],
                        outs=[cc_out.opt()],
                    )
                    n2g = scratch.tile([1, 1], F32, tag="n2g")
                    nc.gpsimd.dma_start(n2g[:], cc_out[:])
                    # squash scale g = sqrt(n2)/(1+n2), n2 = 0.5*allreduced
                    r_t = scratch.tile([1, 1], F32, tag="rt")
                    nc.scalar.activation(r_t[:], n2g[:], AF.Sqrt, scale=0.5)
                    t1_t = scratch.tile([1, 1], F32, tag="t1")
                    nc.vector.tensor_scalar(
                        t1_t[:], n2g[:], 0.5, 1.0, ALU.mult, ALU.add
                    )
                    lt1 = scratch.tile([1, 1], F32, tag="lt1")
                    nc.scalar.activation(lt1[:], t1_t[:], AF.Ln)
                    rt2 = scratch.tile([1, 1], F32, tag="rt2")
                    nc.scalar.activation(rt2[:], lt1[:], AF.Exp, scale=-1.0)
                    g_t = scratch.tile([1, 1], F32, tag="g")
                    nc.vector.tensor_mul(g_t[:], r_t[:], rt2[:])
                    # broadcast g to all partitions via K=1 matmul with ones
                    gb_ps = psmall.tile([P, 1], F32, tag="gb")
                    nc.tensor.matmul(gb_ps[:], onem_sb[:], g_t[:], start=True, stop=True)
                    gb_sb = scratch.tile([P, 1], F32, tag="gbs")
                    nc.any.tensor_copy(gb_sb[:], gb_ps[:])
                    # v = g*s ; W += v
                    v_t = scratch.tile([P, cl, b_dim], F32, tag="v")
                    nc.vector.tensor_scalar_mul(v_t[:], s_t[:], gb_sb[:])
                    nc.vector.tensor_add(w_t[:], w_t[:], v_t[:])

    nc.compile()
    return nc


def prep_inputs(x, w, cl=CL, rh=R // 2, g_batch=G, b_dim=B, ncores=NCORES):
    """Host-side relayout (f32 -> bf16, DMA-friendly order). Returns in_maps."""
    nb = rh // g_batch
    ctot = cl * ncores
    # w: [C, R, I, O] -> [C, 2, NB, I, G, O] bf16
    wb = (
        w.reshape(ctot, 2, nb, g_batch, 64, 64)
        .transpose(0, 1, 2, 4, 3, 5)
        .astype(ml_dtypes.bfloat16)
    )
    # x: [B, C, R, 1, I] -> [C, NB, (2,I)=128, G, B] bf16
    xb = (
        x.reshape(b_dim, ctot, 2, nb, g_batch, 64)
        .transpose(1, 3, 2, 5, 4, 0)
        .reshape(ctot, nb, P, g_batch, b_dim)
        .astype(ml_dtypes.bfloat16)
    )
    f2 = np.equal.outer(np.arange(P) % 64, np.arange(P) % 64).astype(np.float32)
    onek = np.ones((P, 1), np.float32)
    onem = np.ones((1, P), np.float32)
    in_maps = []
    for k in range(ncores):
        in_maps.append(
            {
                "w_in": np.ascontiguousarray(wb[k * cl : (k + 1) * cl]),
                "x_in": np.ascontiguousarray(xb[k * cl : (k + 1) * cl]),
                "f2_in": f2,
                "onek_in": onek,
                "onem_in": onem,
            }
        )
    return in_maps


def postprocess(results, cl=CL, b_dim=B, ncores=NCORES):
    """Fold halves, divide, global squash -> v [B, C, 1, 1, O] f32."""
    ctot = cl * ncores
    s = np.empty((b_dim, ctot, 64), np.float32)
    for k in range(ncores):
        num = np.asarray(results[k]["num_o"], np.float32)  # [P, cl, B]
        den = np.asarray(results[k]["den_o"], np.float32)
        sk = (num[:64] + num[64:]) / (den[:64] + den[64:])  # [64(o), cl, B]
        s[:, k * cl : (k + 1) * cl, :] = sk.transpose(2, 1, 0)
    n2 = np.sum(s.astype(np.float32) ** 2, dtype=np.float32)
    g = np.float32(np.sqrt(n2) / (1.0 + n2))
    v = (g * s).astype(np.float32)
    return v[:, :, None, None, :]


def kernel(x, route_weights, iterations):
    iters = int(iterations)
    assert iters >= 1
    x = np.asarray(x, dtype=np.float32)
    w = np.asarray(route_weights, dtype=np.float32)
    if iters not in _cache:
        _cache[iters] = build(iters)
    nc = _cache[iters]
    in_maps = prep_inputs(x, w)
    res = run_bass_kernel_spmd(
        nc, in_maps, list(range(NCORES)), trace=TRACE, tmpdir=TMPDIR
    )
    LAST_RESULT[0] = res
    return postprocess(res.results)


# revision 25
# speedup vs baseline: 1.6070x; 1.0155x over previous
"""Capsule-routing (ClassCapsLayer) Bass/Tile kernel for 8 trn2 NeuronCores.

Math (reference):
    priors[b,c,r,o] = sum_i x[b,c,r,i] * w[c,r,i,o]
    logits_1 = 0;  logits_{t+1} = logits_t + priors * v_t
    probs_t = softmax_r(logits_t);  s_t = sum_r probs_t * priors
    v_t = squash(s_t)  with GLOBAL Frobenius norm n2 = sum(s_t^2) over (b,c,o)

Key identity: logits_t = priors * W_t with W_t = sum_{u<t} v_u, a per-(b,c,o)
scalar. So each routing iteration needs only one ACT pass
(e = exp(W*priors), fused per-partition scale + fused denominator reduce) and
one DVE pass (tensor_tensor_reduce: numerator = sum_r e*priors), if priors are
laid out with (route-half, o) on partitions and the route index on the free dim.

Matmul: per (class, route-pair) the stationary operand is a 128x128
block-diagonal bf16 weight tile (two 64x64 route weight blocks) -> output
partitions = (half, o), FWL-eligible; moving operand is x [128, B=8].

Sharding: classes split 4-per-core (weights are read exactly once fleet-wide).
The only cross-core quantity is the scalar n2 per iteration -> AllReduce of a
single f32. The final squash is done on the host from per-core partial
numerators/denominators.
"""

import numpy as np
import ml_dtypes

import concourse.bass as bass
import concourse.tile as tile
from concourse import bacc, mybir
from concourse.bass import ts
from concourse.bass_utils import run_bass_kernel_spmd

# Full problem dims (hardcoded; kernel.py must be self-contained)
B, C, R, I, O = 8, 32, 2048, 64, 64
NCORES = 8
CL = C // NCORES      # classes per core
G = 64                # route-pair groups per DMA batch
P = 128

F32 = mybir.dt.float32
BF16 = mybir.dt.bfloat16
AF = mybir.ActivationFunctionType
ALU = mybir.AluOpType

TRACE = False         # set by test.py to collect HW exec time
TMPDIR = None         # set by test.py to keep NTFF/perfetto artifacts
LAST_RESULT = [None]  # BassKernelResults of the most recent run

_cache = {}


def build(iters, cl=CL, rh=R // 2, g_batch=G, b_dim=B, ncores=NCORES):
    """Build the SPMD program. rh = routes/2 (route-pair index range)."""
    nb = rh // g_batch
    nc = bacc.Bacc(
        "TRN2", target_bir_lowering=False, debug=False, num_devices=ncores
    )
    w_in = nc.dram_tensor(
        "w_in", [cl, 2, nb, 64, g_batch, 64], BF16, kind="ExternalInput"
    ).ap()
    x_in = nc.dram_tensor(
        "x_in", [cl, nb, P, g_batch, b_dim], BF16, kind="ExternalInput"
    ).ap()
    f2_in = nc.dram_tensor("f2_in", [P, P], F32, kind="ExternalInput").ap()
    onek_in = nc.dram_tensor("onek_in", [P, 1], F32, kind="ExternalInput").ap()
    onem_in = nc.dram_tensor("onem_in", [1, P], F32, kind="ExternalInput").ap()
    num_o = nc.dram_tensor("num_o", [P, cl, b_dim], F32, kind="ExternalOutput").ap()
    den_o = nc.dram_tensor("den_o", [P, cl, b_dim], F32, kind="ExternalOutput").ap()

    with tile.TileContext(nc) as tc:
        with (
            tc.tile_pool(name="persist", bufs=1) as persist,
            tc.tile_pool(name="wpool", bufs=2) as wpool,
            tc.tile_pool(name="xpool", bufs=3) as xpool,
            tc.tile_pool(name="ppool", bufs=3, space="PSUM") as ppool,
            tc.tile_pool(name="psmall", bufs=1, space="PSUM") as psmall,
            tc.tile_pool(name="scratch", bufs=2) as scratch,
            tc.tile_pool(name="dram", bufs=2, space="DRAM") as dram,
        ):
            # ---- persistent state ----
            # b-major so each (c,b) routing tile is a contiguous [P, rh] slice
            priors = persist.tile([P, cl, b_dim, rh], F32)
            f2_sb = persist.tile([P, P], F32)
            nc.sync.dma_start(f2_sb[:], f2_in[:])
            onek_sb = persist.tile([P, 1], F32)
            nc.sync.dma_start(onek_sb[:], onek_in[:])
            onem_sb = persist.tile([1, P], F32)
            nc.sync.dma_start(onem_sb[:], onem_in[:])
            w_t = persist.tile([P, cl, b_dim], F32)
            nc.vector.memset(w_t[:], 0.0)

            # Two persistent block-diagonal stationary buffers, zeroed once;
            # per-batch DMAs only write the diagonal quadrants, so the
            # off-diagonal zeros persist. Alternating gives double-buffering.
            wb_slots = []
            for si in range(2):
                wbs = persist.tile([P, g_batch, P], BF16, tag=f"wb{si}")
                nc.vector.memset(wbs[:], 0.0)
                wb_slots.append(wbs)

            # ---- priors matmul ----
            # Quadrant DMAs are 128B-line strided; spread them over four
            # HWDGE queues (two per quadrant stream) to parallelize.
            top_eng = [nc.gpsimd, nc.gpsimd]
            bot_eng = [nc.gpsimd, nc.gpsimd]
            for c in range(cl):
                for n in range(nb):
                    bi = c * nb + n
                    wb = wb_slots[bi % 2]
                    top_eng[bi % 2].dma_start(wb[0:64, :, 0:64], w_in[c, 0, n])
                    bot_eng[bi % 2].dma_start(wb[64:128, :, 64:128], w_in[c, 1, n])
                    xs = xpool.tile([P, g_batch, b_dim], BF16, tag="xs")
                    nc.scalar.dma_start(xs[:], x_in[c, n])
                    pt = ppool.tile([P, g_batch, b_dim], F32, tag="pt")
                    for gi in range(g_batch):
                        # out[(h,o), b] = blockdiag_w[(h,i),(h,o)] @ x[(h,i), b]
                        nc.tensor.matmul(
                            pt[:, gi],
                            wb[:, gi, :],
                            xs[:, gi],
                            start=True,
                            stop=True,
                        )
                    nc.vector.tensor_copy(
                        priors[:, c, :, ts(n, g_batch)].rearrange(
                            "p b g -> p g b"
                        ),
                        pt[:],
                    )

            # ---- routing iterations ----
            for it in range(iters):
                num_t = scratch.tile([P, cl, b_dim], F32, tag="num")
                den_t = scratch.tile([P, cl, b_dim], F32, tag="den")
                k = 0
                for c in range(cl):
                    for b in range(b_dim):
                        pr = priors[:, c, b, :]  # [P, rh] contiguous
                        if it == 0:
                            # W == 0 -> e == 1: den is a constant, num is a
                            # plain reduction of priors (split ACT/DVE).
                            if k == 0:
                                nc.vector.memset(den_t[:], float(rh))
                            if k % 2 == 0:
                                nc.vector.tensor_reduce(
                                    num_t[:, c, b : b + 1],
                                    pr,
                                    mybir.AxisListType.X,
                                    ALU.add,
                                )
                            else:
                                sc_t = scratch.tile([P, rh], F32, tag="sc")
                                nc.scalar.activation(
                                    sc_t[:],
                                    pr,
                                    AF.Copy,
                                    accum_out=num_t[:, c, b : b + 1],
                                )
                        else:
                            # e = exp(W * priors); den += sum_r e
                            e_t = scratch.tile([P, rh], F32, tag="e")
                            nc.scalar.activation(
                                e_t[:],
                                pr,
                                AF.Exp,
                                scale=w_t[:, c, b : b + 1],
                                accum_out=den_t[:, c, b : b + 1],
                            )
                            # num = sum_r e * priors (mul on DVE; the
                            # reduction is load-balanced ACT/DVE ~5:3)
                            t_t = scratch.tile([P, rh], F32, tag="tt")
                            nc.vector.tensor_mul(t_t[:], e_t[:], pr)
                            if k % 2 == 0:
                                nc.vector.tensor_reduce(
                                    num_t[:, c, b : b + 1],
                                    t_t[:],
                                    mybir.AxisListType.X,
                                    ALU.add,
                                )
                            else:
                                sc_t = scratch.tile([P, rh], F32, tag="sc")
                                nc.scalar.activation(
                                    sc_t[:],
                                    t_t[:],
                                    AF.Copy,
                                    accum_out=num_t[:, c, b : b + 1],
                                )
                        k += 1
                if it == iters - 1:
                    nc.sync.dma_start(num_o[:], num_t[:])
                    nc.sync.dma_start(den_o[:], den_t[:])
                else:
                    # fold the two route-halves (and duplicate into both
                    # halves) with F2[k,m] = (k%64 == m%64): PE matmul
                    nf = psmall.tile([P, cl, b_dim], F32, tag="nf")
                    df = psmall.tile([P, cl, b_dim], F32, tag="df")
                    nc.tensor.matmul(nf[:], f2_sb[:], num_t[:], start=True, stop=True)
                    nc.tensor.matmul(df[:], f2_sb[:], den_t[:], start=True, stop=True)
                    # 1/den via exp(-ln(den)) (ACT-native; den > 0)
                    ld_t = scratch.tile([P, cl, b_dim], F32, tag="ld")
                    nc.scalar.activation(ld_t[:], df[:], AF.Ln)
                    rd_t = scratch.tile([P, cl, b_dim], F32, tag="rd")
                    nc.scalar.activation(rd_t[:], ld_t[:], AF.Exp, scale=-1.0)
                    s_t = scratch.tile([P, cl, b_dim], F32, tag="s")
                    nc.vector.tensor_mul(s_t[:], nf[:], rd_t[:])
                    # n2_partial = sum(s^2)/2 (each value appears in both halves)
                    sq_t = scratch.tile([P, cl, b_dim], F32, tag="sq")
                    sacc = scratch.tile([P, 1], F32, tag="sacc")
                    nc.scalar.activation(
                        sq_t[:], s_t[:], AF.Square, accum_out=sacc[:]
                    )
                    n2p = psmall.tile([1, 1], F32, tag="n2p")
                    nc.tensor.matmul(n2p[:], onek_sb[:], sacc[:], start=True, stop=True)
                    n2sb = scratch.tile([1, 1], F32, tag="n2sb")
                    nc.any.tensor_copy(n2sb[:], n2p[:])
                    cc_in = dram.tile([1, 1], F32, tag="ccin")
                    cc_out = dram.tile([1, 1], F32, tag="ccout")
                    nc.gpsimd.dma_start(cc_in[:], n2sb[:])
                    nc.gpsimd.collective_compute(
                        "AllReduce",
                        ALU.add,
                        replica_groups=[list(range(ncores))],
                        ins=[cc_in.opt()],
                        outs=[cc_out.opt()],
                    )
                    n2g = scratch.tile([1, 1], F32, tag="n2g")
                    nc.gpsimd.dma_start(n2g[:], cc_out[:])
                    # squash scale g = sqrt(n2)/(1+n2), n2 = 0.5*allreduced
                    r_t = scratch.tile([1, 1], F32, tag="rt")
                    nc.scalar.activation(r_t[:], n2g[:], AF.Sqrt, scale=0.5)
                    t1_t = scratch.tile([1, 1], F32, tag="t1")
                    nc.vector.tensor_scalar(
                        t1_t[:], n2g[:], 0.5, 1.0, ALU.mult, ALU.add
                    )
                    lt1 = scratch.tile([1, 1], F32, tag="lt1")
                    nc.scalar.activation(lt1[:], t1_t[:], AF.Ln)
                    rt2 = scratch.tile([1, 1], F32, tag="rt2")
                    nc.scalar.activation(rt2[:], lt1[:], AF.Exp, scale=-1.0)
                    g_t = scratch.tile([1, 1], F32, tag="g")
                    nc.vector.tensor_mul(g_t[:], r_t[:], rt2[:])
                    # broadcast g to all partitions via K=1 matmul with ones
                    gb_ps = psmall.tile([P, 1], F32, tag="gb")
                    nc.tensor.matmul(gb_ps[:], onem_sb[:], g_t[:], start=True, stop=True)
                    gb_sb = scratch.tile([P, 1], F32, tag="gbs")
                    nc.any.tensor_copy(gb_sb[:], gb_ps[:])
                    # v = g*s ; W += v
                    v_t = scratch.tile([P, cl, b_dim], F32, tag="v")
                    nc.vector.tensor_scalar_mul(v_t[:], s_t[:], gb_sb[:])
                    nc.vector.tensor_add(w_t[:], w_t[:], v_t[:])

    nc.compile()
    return nc


def prep_inputs(x, w, cl=CL, rh=R // 2, g_batch=G, b_dim=B, ncores=NCORES):
    """Host-side relayout (f32 -> bf16, DMA-friendly order). Returns in_maps."""
    nb = rh // g_batch
    ctot = cl * ncores
    # w: [C, R, I, O] -> [C, 2, NB, I, G, O] bf16
    wb = (
        w.reshape(ctot, 2, nb, g_batch, 64, 64)
        .transpose(0, 1, 2, 4, 3, 5)
        .astype(ml_dtypes.bfloat16)
    )
    # x: [B, C, R, 1, I] -> [C, NB, (2,I)=128, G, B] bf16
    xb = (
        x.reshape(b_dim, ctot, 2, nb, g_batch, 64)
        .transpose(1, 3, 2, 5, 4, 0)
        .reshape(ctot, nb, P, g_batch, b_dim)
        .astype(ml_dtypes.bfloat16)
    )
    f2 = np.equal.outer(np.arange(P) % 64, np.arange(P) % 64).astype(np.float32)
    onek = np.ones((P, 1), np.float32)
    onem = np.ones((1, P), np.float32)
    in_maps = []
    for k in range(ncores):
        in_maps.append(
            {
                "w_in": np.ascontiguousarray(wb[k * cl : (k + 1) * cl]),
                "x_in": np.ascontiguousarray(xb[k * cl : (k + 1) * cl]),
                "f2_in": f2,
                "onek_in": onek,
                "onem_in": onem,
            }
        )
    return in_maps


def postprocess(results, cl=CL, b_dim=B, ncores=NCORES):
    """Fold halves, divide, global squash -> v [B, C, 1, 1, O] f32."""
    ctot = cl * ncores
    s = np.empty((b_dim, ctot, 64), np.float32)
    for k in range(ncores):
        num = np.asarray(results[k]["num_o"], np.float32)  # [P, cl, B]
        den = np.asarray(results[k]["den_o"], np.float32)
        sk = (num[:64] + num[64:]) / (den[:64] + den[64:])  # [64(o), cl, B]
        s[:, k * cl : (k + 1) * cl, :] = sk.transpose(2, 1, 0)
    n2 = np.sum(s.astype(np.float32) ** 2, dtype=np.float32)
    g = np.float32(np.sqrt(n2) / (1.0 + n2))
    v = (g * s).astype(np.float32)
    return v[:, :, None, None, :]


def kernel(x, route_weights, iterations):
    iters = int(iterations)
    assert iters >= 1
    x = np.asarray(x, dtype=np.float32)
    w = np.asarray(route_weights, dtype=np.float32)
    if iters not in _cache:
        _cache[iters] = build(iters)
    nc = _cache[iters]
    in_maps = prep_inputs(x, w)
    res = run_bass_kernel_spmd(
        nc, in_maps, list(range(NCORES)), trace=TRACE, tmpdir=TMPDIR
    )
    LAST_RESULT[0] = res
    return postprocess(res.results)
